# revision 1
# baseline (speedup 1.0000x reference)
"""BlockKoopmanNet forward on 8 Trainium2 NeuronCores (Bass/Tile).

Data-parallel over the batch: each core handles B/8 = 2048 rows.
Everything on-device is feature-major ([feature, batch] tiles) so every
layer is a plain lhsT(=weights).T @ rhs(=activations) matmul with no
on-device transposes.  Matmuls run as float32r (TF32-like, 1 cyc/row).

Host-side preprocessing folds all the awkward structure away:
  - x / u are fed pre-transposed (and x zero-padded to 128 rows).
  - u is fed pre-tiled 8x along features for the Bu inner product.
  - The A(x) 2x2 rotation-scale uses column-broadcast copies of the
    a_w3/e_w3 heads so exp/cos/sin and the pair shuffle become pure
    per-partition ops:  cos/sin/-sin are two Sin activations with
    per-partition phase biases (pi/2 shifts), DT is folded into scales.
  - Bu = einsum('bzu,bu->bz', ...) becomes an elementwise multiply with
    the tiled u followed by a 0/1 segment-sum matmul.
  - The output is produced transposed (yT) and un-transposed on host.
"""

import sys

sys.path.insert(0, "/opt/trn_rl_repo")

import numpy as np

DT = 0.02
B, X, U, Z, H, A = 16384, 64, 16, 32, 1024, 256
N_CORES = 8
BC = B // N_CORES  # 2048 rows per core
NB = 512  # batch tile width (matmul free dim)
NCHUNK = BC // NB  # 4

_CACHE = {}


def _build():
    import concourse.bacc as bacc
    import concourse.mybir as mybir
    from concourse.tile import TileContext

    F32 = mybir.dt.float32
    F32R = mybir.dt.float32r
    AF = mybir.ActivationFunctionType
    ALU = mybir.AluOpType

    nc = bacc.Bacc(
        "TRN2", target_bir_lowering=False, debug=False, num_devices=N_CORES
    )

    def din(name, shape, dt=F32R):
        return nc.dram_tensor(name, shape, dt, kind="ExternalInput").ap()

    x2T = din("x2T", (128, BC))
    uR = din("uR", (128, BC), F32)
    w_e1 = din("w_e1", (128, H))
    w_e2 = din("w_e2", (H, H))
    w_z0 = din("w_z0", (H, Z))
    w_z1 = din("w_z1", (H, Z))
    w_a1 = din("w_a1", (128, A))
    w_a2 = din("w_a2", (A, A))
    w_f = din("w_f", (A, Z))
    w_pq = din("w_pq", (A, Z))
    w_b1 = din("w_b1", (128, A))
    w_b2 = din("w_b2", (A, A))
    w_b3 = din("w_b3", (A, Z * U))
    segw = din("segw", (128, 128))
    w_d1 = din("w_d1", (Z, H))
    w_d2 = din("w_d2", (H, H))
    w_d3 = din("w_d3", (H, H))
    w_d4 = din("w_d4", (H, X))
    b_e1 = din("b_e1", (H,), F32)
    b_e2 = din("b_e2", (H,), F32)
    b_z0 = din("b_z0", (Z,), F32)
    b_z1 = din("b_z1", (Z,), F32)
    b_a1 = din("b_a1", (A,), F32)
    b_a2 = din("b_a2", (A,), F32)
    bias_f = din("bias_f", (Z,), F32)
    bias_p = din("bias_p", (Z,), F32)
    bias_q = din("bias_q", (Z,), F32)
    b_b1 = din("b_b1", (A,), F32)
    b_b2 = din("b_b2", (A,), F32)
    b_b3 = din("b_b3", (Z * U,), F32)
    m0 = din("m0", (Z,), F32)
    m1 = din("m1", (Z,), F32)
    b_d1 = din("b_d1", (H,), F32)
    b_d2 = din("b_d2", (H,), F32)
    b_d3 = din("b_d3", (H,), F32)
    b_d4 = din("b_d4", (X,), F32)
    yT = nc.dram_tensor("yT", (X, BC), F32, kind="ExternalOutput").ap()

    with TileContext(nc) as tc:
        with (
            tc.tile_pool(name="wp", bufs=1) as wp,
            tc.tile_pool(name="hp", bufs=2) as hp,
            tc.tile_pool(name="abp", bufs=3) as abp,
            tc.tile_pool(name="xp", bufs=2) as xp,
            tc.tile_pool(name="up", bufs=1) as up,
            tc.tile_pool(name="prp", bufs=2) as prp,
            tc.tile_pool(name="mp", bufs=1) as mp,
            tc.tile_pool(name="znp", bufs=2) as znp,
            tc.tile_pool(name="yp", bufs=1) as yp,
            tc.tile_pool(name="pbig", bufs=4, space="PSUM") as pbig,
            tc.tile_pool(name="pmid", bufs=3, space="PSUM") as pmid,
            tc.tile_pool(name="pyp", bufs=1, space="PSUM") as pyp,
        ):
            def wload(ap, kc, m, tag):
                """Weight [K, M] -> sbuf [128, kc, m] f32r lhsT tile."""
                t = wp.tile([128, kc, m], F32R, tag=tag)
                if kc == 1:
                    nc.sync.dma_start(out=t[:, 0, :], in_=ap)
                else:
                    nc.sync.dma_start(
                        out=t, in_=ap.rearrange("(kc p) m -> p kc m", p=128)
                    )
                return t

            def bload(ap, mc, tag, p=128):
                """Bias [mc*p] -> sbuf [p, mc] f32."""
                t = wp.tile([p, mc], F32, tag=tag)
                if mc == 1:
                    nc.sync.dma_start(out=t[:, 0, None], in_=ap[:, None])
                else:
                    nc.sync.dma_start(
                        out=t, in_=ap.rearrange("(mc p) -> p mc", p=p)
                    )
                return t

            e1w = wload(w_e1, 1, H, "e1w")
            e2w = wload(w_e2, 8, H, "e2w")
            z0w = wload(w_z0, 8, Z, "z0w")
            z1w = wload(w_z1, 8, Z, "z1w")
            a1w = wload(w_a1, 1, A, "a1w")
            a2w = wload(w_a2, 2, A, "a2w")
            fw = wload(w_f, 2, Z, "fw")
            pqw = wload(w_pq, 2, Z, "pqw")
            b1w = wload(w_b1, 1, A, "b1w")
            b2w = wload(w_b2, 2, A, "b2w")
            b3w = wload(w_b3, 2, Z * U, "b3w")
            segt = wp.tile([128, 128], F32R, tag="segt")
            nc.sync.dma_start(out=segt, in_=segw)
            d1w = wp.tile([Z, H], F32R, tag="d1w")
            nc.sync.dma_start(out=d1w, in_=w_d1)
            d2w = wload(w_d2, 8, H, "d2w")
            d3w = wload(w_d3, 8, H, "d3w")
            d4w = wload(w_d4, 8, X, "d4w")

            e1b = bload(b_e1, 8, "e1b")
            e2b = bload(b_e2, 8, "e2b")
            z0b = bload(b_z0, 1, "z0b", p=Z)
            z1b = bload(b_z1, 1, "z1b", p=Z)
            a1b = bload(b_a1, 2, "a1b")
            a2b = bload(b_a2, 2, "a2b")
            fb = bload(bias_f, 1, "fb", p=Z)
            pb = bload(bias_p, 1, "pb", p=Z)
            qb = bload(bias_q, 1, "qb", p=Z)
            b1b = bload(b_b1, 2, "b1b")
            b2b = bload(b_b2, 2, "b2b")
            b3b = bload(b_b3, 4, "b3b")
            m0b = bload(m0, 1, "m0b", p=Z)
            m1b = bload(m1, 1, "m1b", p=Z)
            d1b = bload(b_d1, 8, "d1b")
            d2b = bload(b_d2, 8, "d2b")
            d3b = bload(b_d3, 8, "d3b")
            d4b = bload(b_d4, 1, "d4b", p=X)

            def mlp_layer(w_t, kc, b_t, rhs_fn, h_out, mtiles):
                """h_out[:, m, :] = silu(sum_k w.T @ rhs(k) + b) per m-chunk."""
                for mi in range(mtiles):
                    ps = pbig.tile([128, NB], F32, tag="pb")
                    for k in range(kc):
                        nc.tensor.matmul(
                            ps,
                            w_t[:, k, mi * 128 : (mi + 1) * 128],
                            rhs_fn(k),
                            start=(k == 0),
                            stop=(k == kc - 1),
                        )
                    nc.scalar.activation(
                        h_out[:, mi, :], ps, AF.Silu,
                        bias=b_t[:, mi : mi + 1], scale=1.0,
                    )

            for c in range(NCHUNK):
                cs = c * NB
                x_t = xp.tile([128, NB], F32R, tag="x")
                nc.sync.dma_start(out=x_t, in_=x2T[:, cs : cs + NB])
                u_t = up.tile([128, NB], F32, tag="u")
                nc.sync.dma_start(out=u_t, in_=uR[:, cs : cs + NB])

                # encoder
                h1 = hp.tile([128, 8, NB], F32R, tag="h")
                mlp_layer(e1w, 1, e1b, lambda k: x_t[:], h1, 8)
                h2 = hp.tile([128, 8, NB], F32R, tag="h")
                mlp_layer(e2w, 8, e2b, lambda k: h1[:, k, :], h2, 8)

                # aux head (A(x) params)
                ha1 = abp.tile([128, 2, NB], F32R, tag="ab")
                mlp_layer(a1w, 1, a1b, lambda k: x_t[:], ha1, 2)
                ha2 = abp.tile([128, 2, NB], F32R, tag="ab")
                mlp_layer(a2w, 2, a2b, lambda k: ha1[:, k, :], ha2, 2)

                # B(x) head
                hb1 = abp.tile([128, 2, NB], F32R, tag="ab")
                mlp_layer(b1w, 1, b1b, lambda k: x_t[:], hb1, 2)
                hb2 = abp.tile([128, 2, NB], F32R, tag="ab")
                mlp_layer(b2w, 2, b2b, lambda k: hb1[:, k, :], hb2, 2)

                # F = exp(DT*a), P = [cos|sin](DT*b), Q = [-sin|cos](DT*b)
                # (pair-broadcast rows; phases come in via per-partition bias)
                pfa = pmid.tile([Z, NB], F32, tag="pm")
                for k in range(2):
                    nc.tensor.matmul(
                        pfa, fw[:, k, :], ha2[:, k, :],
                        start=(k == 0), stop=(k == 1),
                    )
                f_t = mp.tile([Z, NB], F32, tag="F")
                nc.scalar.activation(f_t, pfa, AF.Exp, bias=fb[:], scale=DT)

                ppq = pmid.tile([Z, NB], F32, tag="pm")
                for k in range(2):
                    nc.tensor.matmul(
                        ppq, pqw[:, k, :], ha2[:, k, :],
                        start=(k == 0), stop=(k == 1),
                    )
                p_t = mp.tile([Z, NB], F32, tag="P")
                nc.scalar.activation(p_t, ppq, AF.Sin, bias=pb[:], scale=DT)
                q_t = mp.tile([Z, NB], F32, tag="Q")
                nc.scalar.activation(q_t, ppq, AF.Sin, bias=qb[:], scale=DT)

                # z pair-broadcasts
                pz0 = pmid.tile([Z, NB], F32, tag="pm")
                for k in range(8):
                    nc.tensor.matmul(
                        pz0, z0w[:, k, :], h2[:, k, :],
                        start=(k == 0), stop=(k == 7),
                    )
                z0_t = mp.tile([Z, NB], F32, tag="Z0")
                nc.vector.tensor_scalar_add(out=z0_t[:], in0=pz0[:], scalar1=z0b[:])
                pz1 = pmid.tile([Z, NB], F32, tag="pm")
                for k in range(8):
                    nc.tensor.matmul(
                        pz1, z1w[:, k, :], h2[:, k, :],
                        start=(k == 0), stop=(k == 7),
                    )
                z1_t = mp.tile([Z, NB], F32, tag="Z1")
                nc.vector.tensor_scalar_add(out=z1_t[:], in0=pz1[:], scalar1=z1b[:])

                # Bflat + Bu
                prods = []
                for mc in range(4):
                    psb = pbig.tile([128, NB], F32, tag="pb")
                    for k in range(2):
                        nc.tensor.matmul(
                            psb, b3w[:, k, mc * 128 : (mc + 1) * 128],
                            hb2[:, k, :],
                            start=(k == 0), stop=(k == 1),
                        )
                    pr = prp.tile([128, NB], F32R, tag="prod")
                    nc.vector.scalar_tensor_tensor(
                        out=pr[:], in0=psb[:], scalar=b3b[:, mc : mc + 1],
                        in1=u_t[:], op0=ALU.add, op1=ALU.mult,
                    )
                    prods.append(pr)
                pbu = pmid.tile([Z, NB], F32, tag="pm")
                for mc in range(4):
                    nc.tensor.matmul(
                        pbu, segt[:, mc * 32 : (mc + 1) * 32], prods[mc],
                        start=(mc == 0), stop=(mc == 3),
                    )

                # z_next = G0*Z0 + G1*Z1 + DT*Bu   (in-place DVE chain)
                nc.vector.tensor_tensor(
                    out=p_t[:], in0=f_t[:], in1=p_t[:], op=ALU.mult
                )
                nc.vector.tensor_scalar(
                    out=p_t[:], in0=p_t[:], scalar1=DT, scalar2=m0b[:],
                    op0=ALU.mult, op1=ALU.add,
                )
                nc.vector.tensor_tensor(
                    out=q_t[:], in0=f_t[:], in1=q_t[:], op=ALU.mult
                )
                nc.vector.tensor_scalar(
                    out=q_t[:], in0=q_t[:], scalar1=DT, scalar2=m1b[:],
                    op0=ALU.mult, op1=ALU.add,
                )
                nc.vector.tensor_tensor(
                    out=p_t[:], in0=p_t[:], in1=z0_t[:], op=ALU.mult
                )
                nc.vector.tensor_tensor(
                    out=q_t[:], in0=q_t[:], in1=z1_t[:], op=ALU.mult
                )
                nc.vector.tensor_tensor(
                    out=p_t[:], in0=p_t[:], in1=q_t[:], op=ALU.add
                )
                zn_t = znp.tile([Z, NB], F32R, tag="zn")
                nc.vector.scalar_tensor_tensor(
                    out=zn_t[:], in0=pbu[:], scalar=DT, in1=p_t[:],
                    op0=ALU.mult, op1=ALU.add,
                )

                # decoder
                hd1 = hp.tile([128, 8, NB], F32R, tag="h")
                for mi in range(8):
                    ps = pbig.tile([128, NB], F32, tag="pb")
                    nc.tensor.matmul(
                        ps, d1w[:, mi * 128 : (mi + 1) * 128], zn_t[:],
                        start=True, stop=True,
                    )
                    nc.scalar.activation(
                        hd1[:, mi, :], ps, AF.Silu,
                        bias=d1b[:, mi : mi + 1], scale=1.0,
                    )
                hd2 = hp.tile([128, 8, NB], F32R, tag="h")
                mlp_layer(d2w, 8, d2b, lambda k: hd1[:, k, :], hd2, 8)
                hd3 = hp.tile([128, 8, NB], F32R, tag="h")
                mlp_layer(d3w, 8, d3b, lambda k: hd2[:, k, :], hd3, 8)

                py_t = pyp.tile([X, NB], F32, tag="py")
                for k in range(8):
                    nc.tensor.matmul(
                        py_t, d4w[:, k, :], hd3[:, k, :],
                        start=(k == 0), stop=(k == 7),
                    )
                y_sb = yp.tile([X, NB], F32, tag="y")
                nc.vector.tensor_scalar_add(
                    out=y_sb[:], in0=py_t[:], scalar1=d4b[:]
                )
                nc.sync.dma_start(out=yT[:, cs : cs + NB], in_=y_sb)

    nc.compile()
    return nc


def _prep_host(inputs):
    f32 = np.float32
    x = np.asarray(inputs["x"], f32)
    u = np.asarray(inputs["u"], f32)

    x2T = np.zeros((128, B), f32)
    x2T[:X] = x.T
    uR = np.tile(np.ascontiguousarray(u.T), (8, 1))  # [128, B]

    def pad_k(w):
        out = np.zeros((128, w.shape[1]), f32)
        out[: w.shape[0]] = w
        return out

    idx0 = np.arange(Z) // 2 * 2
    idx1 = idx0 + 1
    even = (np.arange(Z) % 2 == 0).astype(f32)

    e_w3 = np.asarray(inputs["e_w3"], f32)
    e_b3 = np.asarray(inputs["e_b3"], f32)
    a_w3 = np.asarray(inputs["a_w3"], f32)
    a_b3 = np.asarray(inputs["a_b3"], f32)

    segw = np.zeros((128, 128), f32)
    for mc in range(4):
        for k in range(128):
            segw[k, mc * 32 + 8 * mc + k // 16] = 1.0

    pi = np.pi
    shared = {
        "w_e1": pad_k(np.asarray(inputs["e_w1"], f32)),
        "b_e1": np.asarray(inputs["e_b1"], f32),
        "w_e2": np.asarray(inputs["e_w2"], f32),
        "b_e2": np.asarray(inputs["e_b2"], f32),
        "w_z0": np.ascontiguousarray(e_w3[:, idx0]),
        "b_z0": np.ascontiguousarray(e_b3[idx0]),
        "w_z1": np.ascontiguousarray(e_w3[:, idx1]),
        "b_z1": np.ascontiguousarray(e_b3[idx1]),
        "w_a1": pad_k(np.asarray(inputs["a_w1"], f32)),
        "b_a1": np.asarray(inputs["a_b1"], f32),
        "w_a2": np.asarray(inputs["a_w2"], f32),
        "b_a2": np.asarray(inputs["a_b2"], f32),
        "w_f": np.ascontiguousarray(a_w3[:, idx0]),
        "w_pq": np.ascontiguousarray(a_w3[:, idx1]),
        "bias_f": (DT * a_b3[idx0]).astype(f32),
        "bias_p": (DT * a_b3[idx1] + even * (pi / 2)).astype(f32),
        "bias_q": (DT * a_b3[idx1] + np.where(even, pi, pi / 2)).astype(f32),
        "w_b1": pad_k(np.asarray(inputs["b_w1"], f32)),
        "b_b1": np.asarray(inputs["b_b1"], f32),
        "w_b2": np.asarray(inputs["b_w2"], f32),
        "b_b2": np.asarray(inputs["b_b2"], f32),
        "w_b3": np.asarray(inputs["b_w3"], f32),
        "b_b3": np.asarray(inputs["b_b3"], f32),
        "segw": segw,
        "m0": even,
        "m1": (1.0 - even).astype(f32),
        "w_d1": np.asarray(inputs["d_w1"], f32),
        "b_d1": np.asarray(inputs["d_b1"], f32),
        "w_d2": np.asarray(inputs["d_w2"], f32),
        "b_d2": np.asarray(inputs["d_b2"], f32),
        "w_d3": np.asarray(inputs["d_w3"], f32),
        "b_d3": np.asarray(inputs["d_b3"], f32),
        "w_d4": np.asarray(inputs["d_w4"], f32),
        "b_d4": np.asarray(inputs["d_b4"], f32),
    }

    in_maps = []
    for c in range(N_CORES):
        sl = slice(c * BC, (c + 1) * BC)
        m = dict(shared)
        m["x2T"] = np.ascontiguousarray(x2T[:, sl])
        m["uR"] = np.ascontiguousarray(uR[:, sl])
        in_maps.append(m)
    return in_maps


def kernel(**inputs) -> np.ndarray:
    from concourse import bass_utils

    if "nc" not in _CACHE:
        _CACHE["nc"] = _build()
    nc = _CACHE["nc"]
    in_maps = _prep_host(inputs)
    res = bass_utils.run_bass_kernel_spmd(
        nc, in_maps, core_ids=list(range(N_CORES))
    )
    return np.concatenate(
        [np.asarray(res.results[c]["yT"]).T for c in range(N_CORES)], axis=0
    ).astype(np.float32)


# revision 7
# speedup vs baseline: 498.4077x; 498.4077x over previous
"""BlockKoopmanNet forward on 8 Trainium2 NeuronCores (Bass/Tile).

Data-parallel over the batch: each core handles B/8 = 2048 rows.
Everything on-device is feature-major ([feature, batch] tiles) so every
layer is a plain lhsT(=weights).T @ rhs(=activations) matmul with no
on-device transposes.  Matmuls run as float32r (TF32-like, 1 cyc/row).

Host-side preprocessing folds all the awkward structure away:
  - x / u are fed pre-transposed; x is fed twice along the partition dim
    so the K=64 input layers run as two row-packed concurrent matmuls.
  - u is fed pre-tiled 8x along features for the Bu inner product.
  - The A(x) 2x2 rotation-scale uses column-broadcast copies of the
    a_w3/e_w3 heads so exp/cos/sin and the pair shuffle become pure
    per-partition ops: cos/sin/-sin are two Sin activations with
    per-partition phase biases (pi/2 shifts), DT is folded into scales.
  - Bu = einsum('bzu,bu->bz', ...) becomes an elementwise multiply with
    the tiled u followed by a 0/1 segment-sum matmul.
  - The output is produced transposed (yT) and un-transposed on host.
"""

import sys

sys.path.insert(0, "/opt/trn_rl_repo")

import numpy as np

DT = 0.02
B, X, U, Z, H, A = 16384, 64, 16, 32, 1024, 256
N_CORES = 8
BC = B // N_CORES  # 2048 rows per core
NB = 512  # batch tile width (matmul free dim)
NCHUNK = BC // NB  # 4

_CACHE = {}


def _build(loop=None):
    import concourse.bacc as bacc
    import concourse.mybir as mybir
    from concourse.tile import TileContext
    from contextlib import nullcontext

    F32 = mybir.dt.float32
    F32R = mybir.dt.float32r
    AF = mybir.ActivationFunctionType
    ALU = mybir.AluOpType

    nc = bacc.Bacc(
        "TRN2", target_bir_lowering=False, debug=False, num_devices=N_CORES
    )

    def din(name, shape, dt=F32R):
        return nc.dram_tensor(name, shape, dt, kind="ExternalInput").ap()

    x2T = din("x2T", (128, BC))
    uR = din("uR", (128, BC), F32)
    w_e1 = din("w_e1", (128, 4, 128))  # row-packed pairs
    w_e2 = din("w_e2", (H, H))
    w_z01 = din("w_z01", (H, 2 * Z))
    w_a1 = din("w_a1", (128, 128))  # row-packed pair
    w_a2 = din("w_a2", (A, A))
    w_fpq = din("w_fpq", (A, 2 * Z))
    w_b1 = din("w_b1", (128, 128))  # row-packed pair
    w_b2 = din("w_b2", (A, A))
    w_b3 = din("w_b3", (A, Z * U))
    segw = din("segw", (128, 128))
    w_d1 = din("w_d1", (Z, H))
    w_d2 = din("w_d2", (H, H))
    w_d3 = din("w_d3", (H, H))
    w_d4 = din("w_d4", (H, X))
    b_e1 = din("b_e1", (H,), F32)
    b_e2 = din("b_e2", (H,), F32)
    b_z01 = din("b_z01", (2 * Z,), F32)
    b_a1 = din("b_a1", (A,), F32)
    b_a2 = din("b_a2", (A,), F32)
    bias_f = din("bias_f", (Z,), F32)
    bias_p = din("bias_p", (Z,), F32)
    bias_q = din("bias_q", (Z,), F32)
    b_b1 = din("b_b1", (A,), F32)
    b_b2 = din("b_b2", (A,), F32)
    b_b3 = din("b_b3", (Z * U,), F32)
    m0 = din("m0", (Z,), F32)
    m1 = din("m1", (Z,), F32)
    b_d1 = din("b_d1", (H,), F32)
    b_d2 = din("b_d2", (H,), F32)
    b_d3 = din("b_d3", (H,), F32)
    b_d4 = din("b_d4", (X,), F32)
    yT = nc.dram_tensor("yT", (X, BC), F32, kind="ExternalOutput").ap()

    with TileContext(nc) as tc:
        with (
            tc.tile_pool(name="wp", bufs=1) as wp,
            tc.tile_pool(name="hp", bufs=2) as hp,
            tc.tile_pool(name="abp", bufs=3) as abp,
            tc.tile_pool(name="xp", bufs=2) as xp,
            tc.tile_pool(name="up", bufs=1) as up,
            tc.tile_pool(name="prp", bufs=2) as prp,
            tc.tile_pool(name="mp", bufs=1) as mp,
            tc.tile_pool(name="znp", bufs=2) as znp,
            tc.tile_pool(name="yp", bufs=1) as yp,
            tc.tile_pool(name="pbig", bufs=4, space="PSUM") as pbig,
            tc.tile_pool(name="pmid", bufs=2, space="PSUM") as pmid,
            tc.tile_pool(name="pbup", bufs=1, space="PSUM") as pbup,
            tc.tile_pool(name="pyp", bufs=1, space="PSUM") as pyp,
        ):
            def wload(ap, kc, m, tag, dma=nc.sync):
                """Weight [K, M] -> sbuf [128, kc, m] f32r lhsT tile."""
                t = wp.tile([128, kc, m], F32R, tag=tag)
                if kc == 1:
                    dma.dma_start(out=t[:, 0, :], in_=ap)
                else:
                    dma.dma_start(
                        out=t, in_=ap.rearrange("(kc p) m -> p kc m", p=128)
                    )
                return t

            def bload(ap, mc, tag, p=128):
                """Bias [mc*p] -> sbuf [p, mc] f32."""
                t = wp.tile([p, mc], F32, tag=tag)
                if mc == 1:
                    nc.sync.dma_start(out=t[:, 0, None], in_=ap[:, None])
                else:
                    nc.sync.dma_start(
                        out=t, in_=ap.rearrange("(mc p) -> p mc", p=p)
                    )
                return t

            # weights in rough usage order; big ones spread across queues
            e1w = wp.tile([128, 4, 128], F32R, tag="e1w")
            nc.sync.dma_start(out=e1w, in_=w_e1)
            a1w = wp.tile([128, 128], F32R, tag="a1w")
            nc.sync.dma_start(out=a1w, in_=w_a1)
            b1w = wp.tile([128, 128], F32R, tag="b1w")
            nc.sync.dma_start(out=b1w, in_=w_b1)
            e2w = wload(w_e2, 8, H, "e2w", dma=nc.scalar)
            a2w = wload(w_a2, 2, A, "a2w")
            b2w = wload(w_b2, 2, A, "b2w")
            fpqw = wload(w_fpq, 2, 2 * Z, "fpqw")
            b3w = wload(w_b3, 2, Z * U, "b3w")
            z01w = wload(w_z01, 8, 2 * Z, "z01w")
            segt = wp.tile([128, 128], F32R, tag="segt")
            nc.sync.dma_start(out=segt, in_=segw)
            d1w = wp.tile([Z, H], F32R, tag="d1w")
            nc.sync.dma_start(out=d1w, in_=w_d1)
            d2w = wload(w_d2, 8, H, "d2w", dma=nc.scalar)
            d3w = wload(w_d3, 8, H, "d3w", dma=nc.gpsimd)
            d4w = wload(w_d4, 8, X, "d4w")

            e1b = bload(b_e1, 8, "e1b")
            e2b = bload(b_e2, 8, "e2b")
            z01b = bload(b_z01, 1, "z01b", p=2 * Z)
            a1b = bload(b_a1, 2, "a1b")
            a2b = bload(b_a2, 2, "a2b")
            fb = bload(bias_f, 1, "fb", p=Z)
            # P/Q biases live at partitions 32-63 so they base-align with
            # the upper half of the fused F|PQ psum tile
            pb = wp.tile([2 * Z, 1], F32, tag="pb")
            nc.sync.dma_start(out=pb[Z:, :], in_=bias_p[:, None])
            qb = wp.tile([2 * Z, 1], F32, tag="qb")
            nc.sync.dma_start(out=qb[Z:, :], in_=bias_q[:, None])
            b1b = bload(b_b1, 2, "b1b")
            b2b = bload(b_b2, 2, "b2b")
            b3b = bload(b_b3, 4, "b3b")
            m0b = bload(m0, 1, "m0b", p=Z)
            m1b = bload(m1, 1, "m1b", p=Z)
            d1b = bload(b_d1, 8, "d1b")
            d2b = bload(b_d2, 8, "d2b")
            d3b = bload(b_d3, 8, "d3b")
            d4b = bload(b_d4, 1, "d4b", p=X)

            def mlp_layer(w_t, kc, b_t, rhs_fn, h_out, mtiles):
                """h_out[:, m, :] = silu(sum_k w.T @ rhs(k) + b) per m-chunk."""
                for mi in range(mtiles):
                    ps = pbig.tile([128, NB], F32, tag="pb")
                    for k in range(kc):
                        nc.tensor.matmul(
                            ps,
                            w_t[:, k, mi * 128 : (mi + 1) * 128],
                            rhs_fn(k),
                            start=(k == 0),
                            stop=(k == kc - 1),
                        )
                    nc.scalar.activation(
                        h_out[:, mi, :], ps, AF.Silu,
                        bias=b_t[:, mi : mi + 1], scale=1.0,
                    )

            def packed_pair(w_pair, x_t, b_t, h_out, j):
                """Two K=64 row-packed concurrent matmuls -> h m-chunks 2j, 2j+1."""
                psa = pbig.tile([128, NB], F32, tag="pb")
                psb = pbig.tile([128, NB], F32, tag="pb")
                nc.tensor.matmul(
                    psa, w_pair[0:64, :], x_t[0:64, :],
                    start=True, stop=True, tile_position=(0, 0),
                )
                nc.tensor.matmul(
                    psb, w_pair[64:128, :], x_t[64:128, :],
                    start=True, stop=True, tile_position=(64, 0),
                )
                nc.scalar.activation(
                    h_out[:, 2 * j, :], psa, AF.Silu,
                    bias=b_t[:, 2 * j : 2 * j + 1], scale=1.0,
                )
                nc.scalar.activation(
                    h_out[:, 2 * j + 1, :], psb, AF.Silu,
                    bias=b_t[:, 2 * j + 1 : 2 * j + 2], scale=1.0,
                )

            loop_ctx = tc.For_i(0, loop, 1) if loop is not None else nullcontext()
            with loop_ctx:
                for c in range(NCHUNK):
                    cs = c * NB
                    x_t = xp.tile([128, NB], F32R, tag="x")
                    nc.sync.dma_start(out=x_t, in_=x2T[:, cs : cs + NB])
                    u_t = up.tile([128, NB], F32, tag="u")
                    nc.sync.dma_start(out=u_t, in_=uR[:, cs : cs + NB])

                    # encoder
                    h1 = hp.tile([128, 8, NB], F32R, tag="h")
                    for j in range(4):
                        packed_pair(e1w[:, j, :], x_t, e1b, h1, j)
                    h2 = hp.tile([128, 8, NB], F32R, tag="h")
                    mlp_layer(e2w, 8, e2b, lambda k: h1[:, k, :], h2, 8)

                    # aux head (A(x) params)
                    ha1 = abp.tile([128, 2, NB], F32R, tag="ab")
                    packed_pair(a1w, x_t, a1b, ha1, 0)
                    ha2 = abp.tile([128, 2, NB], F32R, tag="ab")
                    mlp_layer(a2w, 2, a2b, lambda k: ha1[:, k, :], ha2, 2)

                    # B(x) head
                    hb1 = abp.tile([128, 2, NB], F32R, tag="ab")
                    packed_pair(b1w, x_t, b1b, hb1, 0)
                    hb2 = abp.tile([128, 2, NB], F32R, tag="ab")
                    mlp_layer(b2w, 2, b2b, lambda k: hb1[:, k, :], hb2, 2)

                    # F | P | Q heads: psum [64, NB]; rows 0-31 drive F,
                    # rows 32-63 drive both P and Q (phase-shifted sins)
                    pfpq = pmid.tile([2 * Z, NB], F32, tag="pm")
                    for k in range(2):
                        nc.tensor.matmul(
                            pfpq, fpqw[:, k, :], ha2[:, k, :],
                            start=(k == 0), stop=(k == 1),
                        )
                    f_t = mp.tile([Z, NB], F32, tag="F")
                    nc.scalar.activation(f_t, pfpq[:Z], AF.Exp, bias=fb[:], scale=DT)
                    p_t = mp.tile([Z, NB], F32, tag="P")
                    nc.scalar.activation(p_t, pfpq[Z:], AF.Sin, bias=pb[Z:], scale=DT)
                    q_t = mp.tile([Z, NB], F32, tag="Q")
                    nc.scalar.activation(q_t, pfpq[Z:], AF.Sin, bias=qb[Z:], scale=DT)

                    # z pair-broadcasts Z0|Z1 in one [64, NB] psum
                    pz = pmid.tile([2 * Z, NB], F32, tag="pm")
                    for k in range(8):
                        nc.tensor.matmul(
                            pz, z01w[:, k, :], h2[:, k, :],
                            start=(k == 0), stop=(k == 7),
                        )
                    z0_t = mp.tile([Z, NB], F32, tag="Z0")
                    nc.vector.tensor_scalar_add(
                        out=z0_t[:], in0=pz[:Z], scalar1=z01b[:Z, 0:1]
                    )
                    z1_t = mp.tile([Z, NB], F32, tag="Z1")
                    nc.vector.tensor_scalar_add(
                        out=z1_t[:], in0=pz[Z:], scalar1=z01b[Z:, 0:1]
                    )

                    # Bflat + Bu
                    prods = []
                    for mc in range(4):
                        psb = pbig.tile([128, NB], F32, tag="pb")
                        for k in range(2):
                            nc.tensor.matmul(
                                psb, b3w[:, k, mc * 128 : (mc + 1) * 128],
                                hb2[:, k, :],
                                start=(k == 0), stop=(k == 1),
                            )
                        pr = prp.tile([128, NB], F32R, tag="prod")
                        nc.vector.scalar_tensor_tensor(
                            out=pr[:], in0=psb[:], scalar=b3b[:, mc : mc + 1],
                            in1=u_t[:], op0=ALU.add, op1=ALU.mult,
                        )
                        prods.append(pr)
                    pbu = pbup.tile([Z, NB], F32, tag="pbu")
                    for mc in range(4):
                        nc.tensor.matmul(
                            pbu, segt[:, mc * 32 : (mc + 1) * 32], prods[mc],
                            start=(mc == 0), stop=(mc == 3),
                        )

                    # z_next = G0*Z0 + G1*Z1 + DT*Bu   (in-place DVE chain)
                    nc.vector.tensor_tensor(
                        out=p_t[:], in0=f_t[:], in1=p_t[:], op=ALU.mult
                    )
                    nc.vector.tensor_scalar(
                        out=p_t[:], in0=p_t[:], scalar1=DT, scalar2=m0b[:],
                        op0=ALU.mult, op1=ALU.add,
                    )
                    nc.vector.tensor_tensor(
                        out=q_t[:], in0=f_t[:], in1=q_t[:], op=ALU.mult
                    )
                    nc.vector.tensor_scalar(
                        out=q_t[:], in0=q_t[:], scalar1=DT, scalar2=m1b[:],
                        op0=ALU.mult, op1=ALU.add,
                    )
                    nc.vector.tensor_tensor(
                        out=p_t[:], in0=p_t[:], in1=z0_t[:], op=ALU.mult
                    )
                    nc.vector.tensor_tensor(
                        out=q_t[:], in0=q_t[:], in1=z1_t[:], op=ALU.mult
                    )
                    nc.vector.tensor_tensor(
                        out=p_t[:], in0=p_t[:], in1=q_t[:], op=ALU.add
                    )
                    zn_t = znp.tile([Z, NB], F32R, tag="zn")
                    nc.vector.scalar_tensor_tensor(
                        out=zn_t[:], in0=pbu[:], scalar=DT, in1=p_t[:],
                        op0=ALU.mult, op1=ALU.add,
                    )

                    # decoder
                    hd1 = hp.tile([128, 8, NB], F32R, tag="h")
                    for mi in range(8):
                        ps = pbig.tile([128, NB], F32, tag="pb")
                        nc.tensor.matmul(
                            ps, d1w[:, mi * 128 : (mi + 1) * 128], zn_t[:],
                            start=True, stop=True,
                        )
                        nc.scalar.activation(
                            hd1[:, mi, :], ps, AF.Silu,
                            bias=d1b[:, mi : mi + 1], scale=1.0,
                        )
                    hd2 = hp.tile([128, 8, NB], F32R, tag="h")
                    mlp_layer(d2w, 8, d2b, lambda k: hd1[:, k, :], hd2, 8)
                    hd3 = hp.tile([128, 8, NB], F32R, tag="h")
                    mlp_layer(d3w, 8, d3b, lambda k: hd2[:, k, :], hd3, 8)

                    py_t = pyp.tile([X, NB], F32, tag="py")
                    for k in range(8):
                        nc.tensor.matmul(
                            py_t, d4w[:, k, :], hd3[:, k, :],
                            start=(k == 0), stop=(k == 7),
                        )
                    y_sb = yp.tile([X, NB], F32, tag="y")
                    nc.vector.tensor_scalar_add(
                        out=y_sb[:], in0=py_t[:], scalar1=d4b[:]
                    )
                    nc.sync.dma_start(out=yT[:, cs : cs + NB], in_=y_sb)

    nc.compile()
    return nc


def _prep_host(inputs):
    f32 = np.float32
    x = np.asarray(inputs["x"], f32)
    u = np.asarray(inputs["u"], f32)

    xT = np.ascontiguousarray(x.T)
    x2T = np.concatenate([xT, xT], axis=0)  # [128, B]: x twice (row packing)
    uR = np.tile(np.ascontiguousarray(u.T), (8, 1))  # [128, B]

    def pack_pairs(w):
        """[64, M] -> [128, M//... ] row-packed pairs of 128-col chunks."""
        mt = w.shape[1] // 256
        out = np.zeros((128, mt, 128), f32)
        for j in range(mt):
            out[:64, j] = w[:, (2 * j) * 128 : (2 * j + 1) * 128]
            out[64:, j] = w[:, (2 * j + 1) * 128 : (2 * j + 2) * 128]
        return out

    idx0 = np.arange(Z) // 2 * 2
    idx1 = idx0 + 1
    even = (np.arange(Z) % 2 == 0).astype(f32)

    e_w3 = np.asarray(inputs["e_w3"], f32)
    e_b3 = np.asarray(inputs["e_b3"], f32)
    a_w3 = np.asarray(inputs["a_w3"], f32)
    a_b3 = np.asarray(inputs["a_b3"], f32)

    segw = np.zeros((128, 128), f32)
    for mc in range(4):
        for k in range(128):
            segw[k, mc * 32 + 8 * mc + k // 16] = 1.0

    pi = np.pi
    shared = {
        "w_e1": pack_pairs(np.asarray(inputs["e_w1"], f32)),
        "b_e1": np.asarray(inputs["e_b1"], f32),
        "w_e2": np.asarray(inputs["e_w2"], f32),
        "b_e2": np.asarray(inputs["e_b2"], f32),
        "w_z01": np.ascontiguousarray(
            np.concatenate([e_w3[:, idx0], e_w3[:, idx1]], axis=1)
        ),
        "b_z01": np.ascontiguousarray(
            np.concatenate([e_b3[idx0], e_b3[idx1]])
        ),
        "w_a1": pack_pairs(np.asarray(inputs["a_w1"], f32))[:, 0],
        "b_a1": np.asarray(inputs["a_b1"], f32),
        "w_a2": np.asarray(inputs["a_w2"], f32),
        "b_a2": np.asarray(inputs["a_b2"], f32),
        "w_fpq": np.ascontiguousarray(
            np.concatenate([a_w3[:, idx0], a_w3[:, idx1]], axis=1)
        ),
        "bias_f": (DT * a_b3[idx0]).astype(f32),
        "bias_p": (DT * a_b3[idx1] + even * (pi / 2)).astype(f32),
        "bias_q": (DT * a_b3[idx1] + np.where(even, pi, pi / 2)).astype(f32),
        "w_b1": pack_pairs(np.asarray(inputs["b_w1"], f32))[:, 0],
        "b_b1": np.asarray(inputs["b_b1"], f32),
        "w_b2": np.asarray(inputs["b_w2"], f32),
        "b_b2": np.asarray(inputs["b_b2"], f32),
        "w_b3": np.asarray(inputs["b_w3"], f32),
        "b_b3": np.asarray(inputs["b_b3"], f32),
        "segw": segw,
        "m0": even,
        "m1": (1.0 - even).astype(f32),
        "w_d1": np.asarray(inputs["d_w1"], f32),
        "b_d1": np.asarray(inputs["d_b1"], f32),
        "w_d2": np.asarray(inputs["d_w2"], f32),
        "b_d2": np.asarray(inputs["d_b2"], f32),
        "w_d3": np.asarray(inputs["d_w3"], f32),
        "b_d3": np.asarray(inputs["d_b3"], f32),
        "w_d4": np.asarray(inputs["d_w4"], f32),
        "b_d4": np.asarray(inputs["d_b4"], f32),
    }

    in_maps = []
    for c in range(N_CORES):
        sl = slice(c * BC, (c + 1) * BC)
        m = dict(shared)
        m["x2T"] = np.ascontiguousarray(x2T[:, sl])
        m["uR"] = np.ascontiguousarray(uR[:, sl])
        in_maps.append(m)
    return in_maps


def kernel(**inputs) -> np.ndarray:
    from concourse import bass_utils

    if "nc" not in _CACHE:
        _CACHE["nc"] = _build()
    nc = _CACHE["nc"]
    in_maps = _prep_host(inputs)
    res = bass_utils.run_bass_kernel_spmd(
        nc, in_maps, core_ids=list(range(N_CORES))
    )
    return np.concatenate(
        [np.asarray(res.results[c]["yT"]).T for c in range(N_CORES)], axis=0
    ).astype(np.float32)


# revision 26
# speedup vs baseline: 506.5072x; 1.0163x over previous
"""BlockKoopmanNet forward on 8 Trainium2 NeuronCores (Bass/Tile).

Data-parallel over the batch: each core handles B/8 = 2048 rows.
Everything on-device is feature-major ([feature, batch] tiles) so every
layer is a plain lhsT(=weights).T @ rhs(=activations) matmul with no
on-device transposes.  Matmuls run as float32r (TF32-like, 1 cyc/row).

Host-side preprocessing folds all the awkward structure away:
  - x / u are fed pre-transposed; x is fed twice along the partition dim
    so the K=64 input layers run as two row-packed concurrent matmuls.
  - u is fed pre-tiled 8x along features for the Bu inner product.
  - The A(x) 2x2 rotation-scale uses column-broadcast copies of the
    a_w3/e_w3 heads so exp/cos/sin and the pair shuffle become pure
    per-partition ops: cos/sin/-sin are two Sin activations with
    per-partition phase biases (pi/2 shifts), DT is folded into scales.
  - Bu = einsum('bzu,bu->bz', ...) becomes an elementwise multiply with
    the tiled u followed by a 0/1 segment-sum matmul.
  - The output is produced transposed (yT) and un-transposed on host.
"""

import sys

sys.path.insert(0, "/opt/trn_rl_repo")

import numpy as np

DT = 0.02
B, X, U, Z, H, A = 16384, 64, 16, 32, 1024, 256
N_CORES = 8
BC = B // N_CORES  # 2048 rows per core
NB = 512  # batch tile width (matmul free dim)
NCHUNK = BC // NB  # 4

_CACHE = {}

# column offsets inside the packed small-weight tensor
OFF = {
    "e1": 0,       # 4 pairs x 128
    "a1": 512,
    "b1": 640,
    "a2": 768,     # 2 x 256
    "b2": 1280,
    "fpq": 1792,   # 2 x 64
    "b3": 1920,    # 2 x 512
    "z01": 2944,   # 8 x 64
    "seg": 3456,
    "d4": 3584,    # 8 x 64
    "d1": 4096,    # rows 0-31, 1024 cols
}
WCOLS = 5120
BCOLS = 64


def _build(loop=None):
    import concourse.bacc as bacc
    import concourse.mybir as mybir
    from concourse.tile import TileContext
    from contextlib import nullcontext

    F32 = mybir.dt.float32
    F32R = mybir.dt.float32r
    AF = mybir.ActivationFunctionType
    ALU = mybir.AluOpType

    nc = bacc.Bacc(
        "TRN2", target_bir_lowering=False, debug=False, num_devices=N_CORES
    )

    def din(name, shape, dt=F32R):
        return nc.dram_tensor(name, shape, dt, kind="ExternalInput").ap()

    x2T = din("x2T", (128, BC))
    uR = din("uR", (128, BC), F32)
    # all small weights packed into one per-partition-contiguous tensor
    wpack = din("wpack", (128, WCOLS))
    bpack = din("bpack", (128, BCOLS), F32)
    w_e2 = din("w_e2", (128, 8 * H))
    w_d2 = din("w_d2", (128, 8 * H))
    w_d3 = din("w_d3", (128, 8 * H))
    yT = nc.dram_tensor("yT", (X, BC), F32, kind="ExternalOutput").ap()

    with TileContext(nc) as tc:
        with (
            tc.tile_pool(name="wp", bufs=1) as wp,
            tc.tile_pool(name="hp", bufs=2) as hp,
            tc.tile_pool(name="abp", bufs=3) as abp,
            tc.tile_pool(name="xp", bufs=2) as xp,
            tc.tile_pool(name="up", bufs=2) as up,
            tc.tile_pool(name="prp", bufs=2) as prp,
            tc.tile_pool(name="mp", bufs=1) as mp,
            tc.tile_pool(name="znp", bufs=4) as znp,
            tc.tile_pool(name="yp", bufs=1) as yp,
            tc.tile_pool(name="pbig", bufs=5, space="PSUM") as pbig,
            tc.tile_pool(name="pmid", bufs=2, space="PSUM") as pmid,
            tc.tile_pool(name="pyp", bufs=1, space="PSUM") as pyp,
        ):
            from concourse.tile_rust import add_dep_helper

            # one DMA for all small weights, one for all biases
            wpt = wp.tile([128, WCOLS], F32R, tag="wpt")
            bpt_t = wp.tile([128, BCOLS], F32, tag="bpt")

            def wload(ap, kc, m, tag, dma=nc.sync, dep=None):
                """Host-prearranged flat [128, kc*m] -> sbuf [128, kc, m]."""
                t = wp.tile([128, kc, m], F32R, tag=tag)
                inst = dma.dma_start(out=t[:].rearrange("p kc m -> p (kc m)"), in_=ap)
                if dep is not None:
                    add_dep_helper(inst.ins, dep.ins, reason="weight DMA ordering")
                return t

            # inputs for the first chunks + small weights go FIRST so the
            # input layers are not queued behind 12MB of big weights
            early_xu = {}
            for c in range(2):
                cs = c * NB
                ex = xp.tile([128, NB], F32R, tag="x")
                nc.sync.dma_start(out=ex, in_=x2T[:, cs : cs + NB])
                eu = up.tile([128, NB], F32, tag="u")
                nc.sync.dma_start(out=eu, in_=uR[:, cs : cs + NB])
                early_xu[c] = (ex, eu)

            WSPLIT = OFF["b3"]
            nc.sync.dma_start(out=wpt[:, :WSPLIT], in_=wpack[:, :WSPLIT])
            nc.sync.dma_start(out=bpt_t, in_=bpack)
            i_wp = nc.sync.dma_start(out=wpt[:, WSPLIT:], in_=wpack[:, WSPLIT:])
            bpt = bpt_t[:]

            # big weight matrices: idle gpsimd queue, held behind the small
            # pack (a gated DMA parks its whole issuing queue, so they must
            # not share a queue with compute-critical work)
            e2w = wp.tile([128, 8, H], F32R, tag="e2w")
            w_e2v = w_e2.rearrange("p (k m) -> p k m", k=8)
            i_e2a = nc.gpsimd.dma_start(out=e2w[:, :, : H // 2], in_=w_e2v[:, :, : H // 2])
            add_dep_helper(i_e2a.ins, i_wp.ins, reason="after small weights")
            i_e2b = nc.gpsimd.dma_start(out=e2w[:, :, H // 2 :], in_=w_e2v[:, :, H // 2 :])
            add_dep_helper(i_e2b.ins, i_wp.ins, reason="after small weights")
            d2w = wload(w_d2, 8, H, "d2w", dma=nc.gpsimd, dep=i_wp)
            d3w = wload(w_d3, 8, H, "d3w", dma=nc.gpsimd, dep=i_wp)

            wv = wpt[:]
            e1w = wv[:, OFF["e1"] : OFF["e1"] + 512]
            a1w = wv[:, OFF["a1"] : OFF["a1"] + 128]
            b1w = wv[:, OFF["b1"] : OFF["b1"] + 128]

            class PackedW:
                def __init__(self, name, M):
                    self.name, self.M = name, M

                def __getitem__(self, idx):
                    _, k, ms = idx
                    o = OFF[self.name] + k * self.M
                    lo = ms.start or 0
                    hi = self.M if ms.stop is None else ms.stop
                    return wv[:, o + lo : o + hi]

            a2w = PackedW("a2", A)
            b2w = PackedW("b2", A)
            b3w = PackedW("b3", Z * U)
            z01w = PackedW("z01", 2 * Z)
            fpqw = PackedW("fpq", 2 * Z)
            d4w = PackedW("d4", X)

            segt = wv[:, OFF["seg"] : OFF["seg"] + 128]
            d1w = wv[:32, OFF["d1"] : OFF["d1"] + H]

            e1b = bpt[:, 0:8]
            e2b = bpt[:, 8:16]
            a1b = bpt[:, 16:18]
            a2b = bpt[:, 18:20]
            b1b = bpt[:, 20:22]
            b2b = bpt[:, 22:24]
            b3b = bpt[:, 24:28]
            d1b = bpt[:, 28:36]
            d2b = bpt[:, 36:44]
            d3b = bpt[:, 44:52]
            z01b = bpt[:64, 52:53]
            fb = bpt[:32, 53:54]
            pb_hi = bpt[32:64, 54:55]
            qb_hi = bpt[32:64, 55:56]
            m0b = bpt[:32, 56:57]
            m1b = bpt[:32, 57:58]
            d4b = bpt[:64, 58:59]

            def mlp_layer(w_t, kc, b_t, rhs_fn, h_out, mtiles):
                """h_out[:, m, :] = silu(sum_k w.T @ rhs(k) + b) per m-chunk."""
                for mi in range(mtiles):
                    ps = pbig.tile([128, NB], F32, tag="pb")
                    for k in range(kc):
                        nc.tensor.matmul(
                            ps,
                            w_t[:, k, mi * 128 : (mi + 1) * 128],
                            rhs_fn(k),
                            start=(k == 0),
                            stop=(k == kc - 1),
                        )
                    nc.scalar.activation(
                        h_out[:, mi, :], ps, AF.Silu,
                        bias=b_t[:, mi : mi + 1], scale=1.0,
                    )

            def packed_pair(w_pair, x_t, b_t, h_out, j):
                """Two K=64 row-packed concurrent matmuls -> h m-chunks 2j, 2j+1."""
                psa = pbig.tile([128, NB], F32, tag="pb")
                psb = pbig.tile([128, NB], F32, tag="pb")
                nc.tensor.matmul(
                    psa, w_pair[0:64, :], x_t[0:64, :],
                    start=True, stop=True, tile_position=(0, 0),
                )
                nc.tensor.matmul(
                    psb, w_pair[64:128, :], x_t[64:128, :],
                    start=True, stop=True, tile_position=(64, 0),
                )
                nc.scalar.activation(
                    h_out[:, 2 * j, :], psa, AF.Silu,
                    bias=b_t[:, 2 * j : 2 * j + 1], scale=1.0,
                )
                nc.scalar.activation(
                    h_out[:, 2 * j + 1, :], psb, AF.Silu,
                    bias=b_t[:, 2 * j + 1 : 2 * j + 2], scale=1.0,
                )

            loop_ctx = tc.For_i(0, loop, 1) if loop is not None else nullcontext()
            with loop_ctx:
                zn_tiles = []
                # phase A: encoder + heads + latent step per chunk
                for c in range(NCHUNK):
                    cs = c * NB
                    if loop is None and c in early_xu:
                        x_t, u_t = early_xu[c]
                    else:
                        x_t = xp.tile([128, NB], F32R, tag="x")
                        nc.sync.dma_start(out=x_t, in_=x2T[:, cs : cs + NB])
                        u_t = up.tile([128, NB], F32, tag="u")
                        nc.sync.dma_start(out=u_t, in_=uR[:, cs : cs + NB])

                    # input layers + small heads first (only need x + small
                    # weights), so the e2w stream can still be in flight
                    h1 = hp.tile([128, 8, NB], F32R, tag="h")
                    for j in range(4):
                        packed_pair(e1w[:, j * 128 : (j + 1) * 128], x_t, e1b, h1, j)

                    # aux head (A(x) params)
                    ha1 = abp.tile([128, 2, NB], F32R, tag="ab")
                    packed_pair(a1w, x_t, a1b, ha1, 0)
                    ha2 = abp.tile([128, 2, NB], F32R, tag="ab")
                    mlp_layer(a2w, 2, a2b, lambda k: ha1[:, k, :], ha2, 2)

                    # B(x) head
                    hb1 = abp.tile([128, 2, NB], F32R, tag="ab")
                    packed_pair(b1w, x_t, b1b, hb1, 0)
                    hb2 = abp.tile([128, 2, NB], F32R, tag="ab")
                    mlp_layer(b2w, 2, b2b, lambda k: hb1[:, k, :], hb2, 2)

                    # big encoder layer: PE grinds here while ACT catches up
                    # on the head silus and the F/P/Q activations
                    h2 = hp.tile([128, 8, NB], F32R, tag="h")
                    mlp_layer(e2w, 8, e2b, lambda k: h1[:, k, :], h2, 8)

                    # F | P | Q heads: psum [64, NB]; rows 0-31 drive F,
                    # rows 32-63 drive both P and Q (phase-shifted sins)
                    pfpq = pmid.tile([2 * Z, NB], F32, tag="pm")
                    for k in range(2):
                        nc.tensor.matmul(
                            pfpq, fpqw[:, k, :], ha2[:, k, :],
                            start=(k == 0), stop=(k == 1),
                        )
                    f_t = mp.tile([Z, NB], F32, tag="F")
                    nc.scalar.activation(f_t, pfpq[:Z], AF.Exp, bias=fb, scale=DT)
                    p_t = mp.tile([Z, NB], F32, tag="P")
                    nc.scalar.activation(p_t, pfpq[Z:], AF.Sin, bias=pb_hi, scale=DT)
                    q_t = mp.tile([Z, NB], F32, tag="Q")
                    nc.scalar.activation(q_t, pfpq[Z:], AF.Sin, bias=qb_hi, scale=DT)

                    # z pair-broadcasts Z0|Z1 in one [64, NB] psum
                    pz = pmid.tile([2 * Z, NB], F32, tag="pm")
                    for k in range(8):
                        nc.tensor.matmul(
                            pz, z01w[:, k, :], h2[:, k, :],
                            start=(k == 0), stop=(k == 7),
                        )
                    z0_t = mp.tile([Z, NB], F32, tag="Z0")
                    nc.vector.tensor_scalar_add(
                        out=z0_t[:], in0=pz[:Z], scalar1=z01b[:Z, 0:1]
                    )
                    z1_t = mp.tile([Z, NB], F32, tag="Z1")
                    nc.vector.tensor_scalar_add(
                        out=z1_t[:], in0=pz[Z:], scalar1=z01b[Z:, 0:1]
                    )

                    # Bflat + Bu
                    prods = []
                    for mc in range(4):
                        psb = pbig.tile([128, NB], F32, tag="pb")
                        for k in range(2):
                            nc.tensor.matmul(
                                psb, b3w[:, k, mc * 128 : (mc + 1) * 128],
                                hb2[:, k, :],
                                start=(k == 0), stop=(k == 1),
                            )
                        pr = prp.tile([128, NB], F32R, tag="prod")
                        nc.vector.scalar_tensor_tensor(
                            out=pr[:], in0=psb[:], scalar=b3b[:, mc : mc + 1],
                            in1=u_t[:], op0=ALU.add, op1=ALU.mult,
                        )
                        prods.append(pr)
                    pbu = pmid.tile([Z, NB], F32, tag="pm")
                    for mc in range(4):
                        nc.tensor.matmul(
                            pbu, segt[:, mc * 32 : (mc + 1) * 32], prods[mc],
                            start=(mc == 0), stop=(mc == 3),
                        )

                    # z_next = G0*Z0 + G1*Z1 + DT*Bu   (in-place DVE chain)
                    nc.vector.tensor_tensor(
                        out=p_t[:], in0=f_t[:], in1=p_t[:], op=ALU.mult
                    )
                    nc.vector.tensor_scalar(
                        out=p_t[:], in0=p_t[:], scalar1=DT, scalar2=m0b,
                        op0=ALU.mult, op1=ALU.add,
                    )
                    nc.vector.tensor_tensor(
                        out=q_t[:], in0=f_t[:], in1=q_t[:], op=ALU.mult
                    )
                    nc.vector.tensor_scalar(
                        out=q_t[:], in0=q_t[:], scalar1=DT, scalar2=m1b,
                        op0=ALU.mult, op1=ALU.add,
                    )
                    nc.vector.tensor_tensor(
                        out=p_t[:], in0=p_t[:], in1=z0_t[:], op=ALU.mult
                    )
                    nc.vector.tensor_tensor(
                        out=q_t[:], in0=q_t[:], in1=z1_t[:], op=ALU.mult
                    )
                    nc.vector.tensor_tensor(
                        out=p_t[:], in0=p_t[:], in1=q_t[:], op=ALU.add
                    )
                    zn_t = znp.tile([Z, NB], F32R, tag="zn")
                    nc.vector.scalar_tensor_tensor(
                        out=zn_t[:], in0=pbu[:], scalar=DT, in1=p_t[:],
                        op0=ALU.mult, op1=ALU.add,
                    )
                    zn_tiles.append(zn_t)

                # phase B: decoders, pipelined behind phase A on PE
                for c in range(NCHUNK):
                    cs = c * NB
                    zn_t = zn_tiles[c]
                    hd1 = hp.tile([128, 8, NB], F32R, tag="h")
                    for mi in range(8):
                        ps = pbig.tile([128, NB], F32, tag="pb")
                        nc.tensor.matmul(
                            ps, d1w[:, mi * 128 : (mi + 1) * 128], zn_t[:],
                            start=True, stop=True,
                        )
                        nc.scalar.activation(
                            hd1[:, mi, :], ps, AF.Silu,
                            bias=d1b[:, mi : mi + 1], scale=1.0,
                        )
                    hd2 = hp.tile([128, 8, NB], F32R, tag="h")
                    mlp_layer(d2w, 8, d2b, lambda k: hd1[:, k, :], hd2, 8)
                    hd3 = hp.tile([128, 8, NB], F32R, tag="h")
                    mlp_layer(d3w, 8, d3b, lambda k: hd2[:, k, :], hd3, 8)

                    py_t = pyp.tile([X, NB], F32, tag="py")
                    for k in range(8):
                        nc.tensor.matmul(
                            py_t, d4w[:, k, :], hd3[:, k, :],
                            start=(k == 0), stop=(k == 7),
                        )
                    y_sb = yp.tile([X, NB], F32, tag="y")
                    nc.vector.tensor_scalar_add(
                        out=y_sb[:], in0=py_t[:], scalar1=d4b
                    )
                    nc.sync.dma_start(out=yT[:, cs : cs + NB], in_=y_sb)

    nc.compile()
    return nc


def _prep_host(inputs):
    f32 = np.float32
    x = np.asarray(inputs["x"], f32)
    u = np.asarray(inputs["u"], f32)

    xT = np.ascontiguousarray(x.T)
    x2T = np.concatenate([xT, xT], axis=0)  # [128, B]: x twice (row packing)
    uR = np.tile(np.ascontiguousarray(u.T), (8, 1))  # [128, B]

    def fm(w):
        """[K, M] -> [128, (K//128)*M]: per-partition-contiguous lhsT chunks."""
        kc = w.shape[0] // 128
        return np.ascontiguousarray(
            w.reshape(kc, 128, w.shape[1]).transpose(1, 0, 2).reshape(128, -1)
        )

    def pack_pairs(w):
        """[64, M] -> [128, M//... ] row-packed pairs of 128-col chunks."""
        mt = w.shape[1] // 256
        out = np.zeros((128, mt, 128), f32)
        for j in range(mt):
            out[:64, j] = w[:, (2 * j) * 128 : (2 * j + 1) * 128]
            out[64:, j] = w[:, (2 * j + 1) * 128 : (2 * j + 2) * 128]
        return out

    idx0 = np.arange(Z) // 2 * 2
    idx1 = idx0 + 1
    even = (np.arange(Z) % 2 == 0).astype(f32)

    e_w3 = np.asarray(inputs["e_w3"], f32)
    e_b3 = np.asarray(inputs["e_b3"], f32)
    a_w3 = np.asarray(inputs["a_w3"], f32)
    a_b3 = np.asarray(inputs["a_b3"], f32)

    segw = np.zeros((128, 128), f32)
    for mc in range(4):
        for k in range(128):
            segw[k, mc * 32 + 8 * mc + k // 16] = 1.0

    pi = np.pi

    wpack = np.zeros((128, WCOLS), f32)
    wpack[:, OFF["e1"] : OFF["e1"] + 512] = pack_pairs(
        np.asarray(inputs["e_w1"], f32)
    ).reshape(128, 512)
    wpack[:, OFF["a1"] : OFF["a1"] + 128] = pack_pairs(
        np.asarray(inputs["a_w1"], f32)
    )[:, 0]
    wpack[:, OFF["b1"] : OFF["b1"] + 128] = pack_pairs(
        np.asarray(inputs["b_w1"], f32)
    )[:, 0]
    wpack[:, OFF["a2"] : OFF["a2"] + 512] = fm(np.asarray(inputs["a_w2"], f32))
    wpack[:, OFF["b2"] : OFF["b2"] + 512] = fm(np.asarray(inputs["b_w2"], f32))
    wpack[:, OFF["fpq"] : OFF["fpq"] + 128] = fm(
        np.concatenate([a_w3[:, idx0], a_w3[:, idx1]], axis=1)
    )
    wpack[:, OFF["b3"] : OFF["b3"] + 1024] = fm(np.asarray(inputs["b_w3"], f32))
    wpack[:, OFF["z01"] : OFF["z01"] + 512] = fm(
        np.concatenate([e_w3[:, idx0], e_w3[:, idx1]], axis=1)
    )
    wpack[:, OFF["seg"] : OFF["seg"] + 128] = segw
    wpack[:, OFF["d4"] : OFF["d4"] + 512] = fm(np.asarray(inputs["d_w4"], f32))
    wpack[:32, OFF["d1"] : OFF["d1"] + H] = np.asarray(inputs["d_w1"], f32)

    def bcol(b):
        return np.asarray(b, f32).reshape(-1, 128).T

    bpack = np.zeros((128, BCOLS), f32)
    bpack[:, 0:8] = bcol(inputs["e_b1"])
    bpack[:, 8:16] = bcol(inputs["e_b2"])
    bpack[:, 16:18] = bcol(inputs["a_b1"])
    bpack[:, 18:20] = bcol(inputs["a_b2"])
    bpack[:, 20:22] = bcol(inputs["b_b1"])
    bpack[:, 22:24] = bcol(inputs["b_b2"])
    bpack[:, 24:28] = bcol(inputs["b_b3"])
    bpack[:, 28:36] = bcol(inputs["d_b1"])
    bpack[:, 36:44] = bcol(inputs["d_b2"])
    bpack[:, 44:52] = bcol(inputs["d_b3"])
    bpack[:64, 52] = np.concatenate([e_b3[idx0], e_b3[idx1]])
    bpack[:32, 53] = DT * a_b3[idx0]
    bpack[32:64, 54] = DT * a_b3[idx1] + even * (pi / 2)
    bpack[32:64, 55] = DT * a_b3[idx1] + np.where(even, pi, pi / 2)
    bpack[:32, 56] = even
    bpack[:32, 57] = 1.0 - even
    bpack[:64, 58] = np.asarray(inputs["d_b4"], f32)

    shared = {
        "wpack": wpack,
        "bpack": bpack,
        "w_e2": fm(np.asarray(inputs["e_w2"], f32)),
        "w_d2": fm(np.asarray(inputs["d_w2"], f32)),
        "w_d3": fm(np.asarray(inputs["d_w3"], f32)),
    }

    in_maps = []
    for c in range(N_CORES):
        sl = slice(c * BC, (c + 1) * BC)
        m = dict(shared)
        m["x2T"] = np.ascontiguousarray(x2T[:, sl])
        m["uR"] = np.ascontiguousarray(uR[:, sl])
        in_maps.append(m)
    return in_maps


def kernel(**inputs) -> np.ndarray:
    from concourse import bass_utils

    if "nc" not in _CACHE:
        _CACHE["nc"] = _build()
    nc = _CACHE["nc"]
    in_maps = _prep_host(inputs)
    res = bass_utils.run_bass_kernel_spmd(
        nc, in_maps, core_ids=list(range(N_CORES))
    )
    return np.concatenate(
        [np.asarray(res.results[c]["yT"]).T for c in range(N_CORES)], axis=0
    ).astype(np.float32)


# revision 30
# speedup vs baseline: 544.7167x; 1.0754x over previous
"""BlockKoopmanNet forward on 8 Trainium2 NeuronCores (Bass/Tile).

Data-parallel over the batch: each core handles B/8 = 2048 rows.
Everything on-device is feature-major ([feature, batch] tiles) so every
layer is a plain lhsT(=weights).T @ rhs(=activations) matmul with no
on-device transposes.  Matmuls run as float32r (TF32-like, 1 cyc/row).

Host-side preprocessing folds all the awkward structure away:
  - x / u are fed pre-transposed; x is fed twice along the partition dim
    so the K=64 input layers run as two row-packed concurrent matmuls.
  - u is fed pre-tiled 8x along features for the Bu inner product.
  - The A(x) 2x2 rotation-scale uses column-broadcast copies of the
    a_w3/e_w3 heads so exp/cos/sin and the pair shuffle become pure
    per-partition ops: cos/sin/-sin are two Sin activations with
    per-partition phase biases (pi/2 shifts), DT is folded into scales.
  - Bu = einsum('bzu,bu->bz', ...) becomes an elementwise multiply with
    the tiled u followed by a 0/1 segment-sum matmul.
  - The output is produced transposed (yT) and un-transposed on host.
"""

import sys

sys.path.insert(0, "/opt/trn_rl_repo")

import numpy as np

DT = 0.02
B, X, U, Z, H, A = 16384, 64, 16, 32, 1024, 256
N_CORES = 8
BC = B // N_CORES  # 2048 rows per core
NB = 512  # batch tile width (matmul free dim)
NCHUNK = BC // NB  # 4

_CACHE = {}

# column offsets inside the packed small-weight tensor
OFF = {
    "e1": 0,       # 4 pairs x 128
    "a1": 512,
    "b1": 640,
    "a2": 768,     # 2 x 256
    "b2": 1280,
    "fpq": 1792,   # 2 x 64
    "b3": 1920,    # 2 x 512
    "z01": 2944,   # 8 x 64
    "seg": 3456,
    "d4": 3584,    # 8 x 64
    "d1": 4096,    # quad-packed: rows 32r..32r+32, col-group g, m = 4g+r
}
WCOLS = 4352
BCOLS = 64


def _build(loop=None):
    import concourse.bacc as bacc
    import concourse.mybir as mybir
    from concourse.tile import TileContext
    from contextlib import nullcontext

    F32 = mybir.dt.float32
    F32R = mybir.dt.float32r
    AF = mybir.ActivationFunctionType
    ALU = mybir.AluOpType

    nc = bacc.Bacc(
        "TRN2", target_bir_lowering=False, debug=False, num_devices=N_CORES
    )

    def din(name, shape, dt=F32R):
        return nc.dram_tensor(name, shape, dt, kind="ExternalInput").ap()

    x2T = din("x2T", (128, BC))
    uR = din("uR", (128, BC), F32)
    # all small weights packed into one per-partition-contiguous tensor
    wpack = din("wpack", (128, WCOLS))
    bpack = din("bpack", (128, BCOLS), F32)
    w_e2 = din("w_e2", (128, 8 * H))
    w_d2 = din("w_d2", (128, 8 * H))
    w_d3 = din("w_d3", (128, 8 * H))
    yT = nc.dram_tensor("yT", (X, BC), F32, kind="ExternalOutput").ap()

    with TileContext(nc) as tc:
        with (
            tc.tile_pool(name="wp", bufs=1) as wp,
            tc.tile_pool(name="hp", bufs=2) as hp,
            tc.tile_pool(name="abp", bufs=3) as abp,
            tc.tile_pool(name="xp", bufs=2) as xp,
            tc.tile_pool(name="up", bufs=2) as up,
            tc.tile_pool(name="prp", bufs=2) as prp,
            tc.tile_pool(name="mp", bufs=1) as mp,
            tc.tile_pool(name="znp", bufs=4) as znp,
            tc.tile_pool(name="yp", bufs=1) as yp,
            tc.tile_pool(name="pbig", bufs=5, space="PSUM") as pbig,
            tc.tile_pool(name="pmid", bufs=2, space="PSUM") as pmid,
            tc.tile_pool(name="pyp", bufs=1, space="PSUM") as pyp,
        ):
            from concourse.tile_rust import add_dep_helper

            # one DMA for all small weights, one for all biases
            wpt = wp.tile([128, WCOLS], F32R, tag="wpt")
            bpt_t = wp.tile([128, BCOLS], F32, tag="bpt")

            def wload(ap, kc, m, tag, dma=nc.sync, dep=None):
                """Host-prearranged flat [128, kc*m] -> sbuf [128, kc, m]."""
                t = wp.tile([128, kc, m], F32R, tag=tag)
                inst = dma.dma_start(out=t[:].rearrange("p kc m -> p (kc m)"), in_=ap)
                if dep is not None:
                    add_dep_helper(inst.ins, dep.ins, reason="weight DMA ordering")
                return t

            # inputs for the first chunks + small weights go FIRST so the
            # input layers are not queued behind 12MB of big weights
            early_xu = {}
            for c in range(2):
                cs = c * NB
                ex = xp.tile([128, NB], F32R, tag="x")
                nc.sync.dma_start(out=ex, in_=x2T[:, cs : cs + NB])
                eu = up.tile([128, NB], F32, tag="u")
                nc.sync.dma_start(out=eu, in_=uR[:, cs : cs + NB])
                early_xu[c] = (ex, eu)

            WSPLIT = OFF["b3"]
            nc.sync.dma_start(out=wpt[:, :WSPLIT], in_=wpack[:, :WSPLIT])
            nc.sync.dma_start(out=bpt_t, in_=bpack)
            i_wp = nc.sync.dma_start(out=wpt[:, WSPLIT:], in_=wpack[:, WSPLIT:])
            bpt = bpt_t[:]

            # big weight matrices: idle gpsimd queue, held behind the small
            # pack (a gated DMA parks its whole issuing queue, so they must
            # not share a queue with compute-critical work)
            e2w = wp.tile([128, 8, H], F32R, tag="e2w")
            w_e2v = w_e2.rearrange("p (k m) -> p k m", k=8)
            i_e2a = nc.gpsimd.dma_start(out=e2w[:, :, : H // 2], in_=w_e2v[:, :, : H // 2])
            add_dep_helper(i_e2a.ins, i_wp.ins, reason="after small weights")
            i_e2b = nc.gpsimd.dma_start(out=e2w[:, :, H // 2 :], in_=w_e2v[:, :, H // 2 :])
            add_dep_helper(i_e2b.ins, i_wp.ins, reason="after small weights")
            d2w = wload(w_d2, 8, H, "d2w", dma=nc.gpsimd, dep=i_wp)
            d3w = wload(w_d3, 8, H, "d3w", dma=nc.gpsimd, dep=i_wp)

            wv = wpt[:]
            e1w = wv[:, OFF["e1"] : OFF["e1"] + 512]
            a1w = wv[:, OFF["a1"] : OFF["a1"] + 128]
            b1w = wv[:, OFF["b1"] : OFF["b1"] + 128]

            class PackedW:
                def __init__(self, name, M):
                    self.name, self.M = name, M

                def __getitem__(self, idx):
                    _, k, ms = idx
                    o = OFF[self.name] + k * self.M
                    lo = ms.start or 0
                    hi = self.M if ms.stop is None else ms.stop
                    return wv[:, o + lo : o + hi]

            a2w = PackedW("a2", A)
            b2w = PackedW("b2", A)
            b3w = PackedW("b3", Z * U)
            z01w = PackedW("z01", 2 * Z)
            fpqw = PackedW("fpq", 2 * Z)
            d4w = PackedW("d4", X)

            segt = wv[:, OFF["seg"] : OFF["seg"] + 128]
            d1w = wv[:, OFF["d1"] : OFF["d1"] + 256]

            e1b = bpt[:, 0:8]
            e2b = bpt[:, 8:16]
            a1b = bpt[:, 16:18]
            a2b = bpt[:, 18:20]
            b1b = bpt[:, 20:22]
            b2b = bpt[:, 22:24]
            b3b = bpt[:, 24:28]
            d1b = bpt[:, 28:36]
            d2b = bpt[:, 36:44]
            d3b = bpt[:, 44:52]
            z01b = bpt[:64, 52:53]
            fb = bpt[:32, 53:54]
            pb_hi = bpt[32:64, 54:55]
            qb_hi = bpt[32:64, 55:56]
            m0b = bpt[:32, 56:57]
            m1b = bpt[:32, 57:58]
            d4b = bpt[:64, 58:59]

            def mlp_layer(w_t, kc, b_t, rhs_fn, h_out, mtiles):
                """h_out[:, m, :] = silu(sum_k w.T @ rhs(k) + b) per m-chunk."""
                for mi in range(mtiles):
                    ps = pbig.tile([128, NB], F32, tag="pb")
                    for k in range(kc):
                        nc.tensor.matmul(
                            ps,
                            w_t[:, k, mi * 128 : (mi + 1) * 128],
                            rhs_fn(k),
                            start=(k == 0),
                            stop=(k == kc - 1),
                        )
                    nc.scalar.activation(
                        h_out[:, mi, :], ps, AF.Silu,
                        bias=b_t[:, mi : mi + 1], scale=1.0,
                    )

            def packed_pair(w_pair, x_t, b_t, h_out, j):
                """Two K=64 row-packed concurrent matmuls -> h m-chunks 2j, 2j+1."""
                psa = pbig.tile([128, NB], F32, tag="pb")
                psb = pbig.tile([128, NB], F32, tag="pb")
                nc.tensor.matmul(
                    psa, w_pair[0:64, :], x_t[0:64, :],
                    start=True, stop=True, tile_position=(0, 0),
                )
                nc.tensor.matmul(
                    psb, w_pair[64:128, :], x_t[64:128, :],
                    start=True, stop=True, tile_position=(64, 0),
                )
                nc.scalar.activation(
                    h_out[:, 2 * j, :], psa, AF.Silu,
                    bias=b_t[:, 2 * j : 2 * j + 1], scale=1.0,
                )
                nc.scalar.activation(
                    h_out[:, 2 * j + 1, :], psb, AF.Silu,
                    bias=b_t[:, 2 * j + 1 : 2 * j + 2], scale=1.0,
                )

            loop_ctx = tc.For_i(0, loop, 1) if loop is not None else nullcontext()
            with loop_ctx:
                zn_tiles = []
                # phase A: encoder + heads + latent step per chunk
                for c in range(NCHUNK):
                    cs = c * NB
                    if loop is None and c in early_xu:
                        x_t, u_t = early_xu[c]
                    else:
                        x_t = xp.tile([128, NB], F32R, tag="x")
                        nc.sync.dma_start(out=x_t, in_=x2T[:, cs : cs + NB])
                        u_t = up.tile([128, NB], F32, tag="u")
                        nc.sync.dma_start(out=u_t, in_=uR[:, cs : cs + NB])

                    # input layers + small heads first (only need x + small
                    # weights), so the e2w stream can still be in flight
                    h1 = hp.tile([128, 8, NB], F32R, tag="h")
                    for j in range(4):
                        packed_pair(e1w[:, j * 128 : (j + 1) * 128], x_t, e1b, h1, j)

                    # aux head (A(x) params)
                    ha1 = abp.tile([128, 2, NB], F32R, tag="ab")
                    packed_pair(a1w, x_t, a1b, ha1, 0)
                    ha2 = abp.tile([128, 2, NB], F32R, tag="ab")
                    mlp_layer(a2w, 2, a2b, lambda k: ha1[:, k, :], ha2, 2)

                    # B(x) head
                    hb1 = abp.tile([128, 2, NB], F32R, tag="ab")
                    packed_pair(b1w, x_t, b1b, hb1, 0)
                    hb2 = abp.tile([128, 2, NB], F32R, tag="ab")
                    mlp_layer(b2w, 2, b2b, lambda k: hb1[:, k, :], hb2, 2)

                    # big encoder layer, with the F|P|Q head block slotted
                    # in after two m-chunks: the Exp/Sin table loads then
                    # happen while PE grinds e2, not at the chunk boundary
                    h2 = hp.tile([128, 8, NB], F32R, tag="h")
                    mlp_layer(e2w, 8, e2b, lambda k: h1[:, k, :], h2, 2)

                    # F | P | Q heads: psum [64, NB]; rows 0-31 drive F,
                    # rows 32-63 drive both P and Q (phase-shifted sins)
                    pfpq = pmid.tile([2 * Z, NB], F32, tag="pm")
                    for k in range(2):
                        nc.tensor.matmul(
                            pfpq, fpqw[:, k, :], ha2[:, k, :],
                            start=(k == 0), stop=(k == 1),
                        )
                    f_t = mp.tile([Z, NB], F32, tag="F")
                    nc.scalar.activation(f_t, pfpq[:Z], AF.Exp, bias=fb, scale=DT)
                    p_t = mp.tile([Z, NB], F32, tag="P")
                    nc.scalar.activation(p_t, pfpq[Z:], AF.Sin, bias=pb_hi, scale=DT)
                    q_t = mp.tile([Z, NB], F32, tag="Q")
                    nc.scalar.activation(q_t, pfpq[Z:], AF.Sin, bias=qb_hi, scale=DT)

                    for mi in range(2, 8):
                        ps = pbig.tile([128, NB], F32, tag="pb")
                        for k in range(8):
                            nc.tensor.matmul(
                                ps, e2w[:, k, mi * 128 : (mi + 1) * 128],
                                h1[:, k, :],
                                start=(k == 0), stop=(k == 7),
                            )
                        nc.scalar.activation(
                            h2[:, mi, :], ps, AF.Silu,
                            bias=e2b[:, mi : mi + 1], scale=1.0,
                        )

                    # z pair-broadcasts Z0|Z1 in one [64, NB] psum
                    pz = pmid.tile([2 * Z, NB], F32, tag="pm")
                    for k in range(8):
                        nc.tensor.matmul(
                            pz, z01w[:, k, :], h2[:, k, :],
                            start=(k == 0), stop=(k == 7),
                        )
                    z0_t = mp.tile([Z, NB], F32, tag="Z0")
                    nc.vector.tensor_scalar_add(
                        out=z0_t[:], in0=pz[:Z], scalar1=z01b[:Z, 0:1]
                    )
                    z1_t = mp.tile([Z, NB], F32, tag="Z1")
                    nc.vector.tensor_scalar_add(
                        out=z1_t[:], in0=pz[Z:], scalar1=z01b[Z:, 0:1]
                    )

                    # Bflat + Bu
                    prods = []
                    for mc in range(4):
                        psb = pbig.tile([128, NB], F32, tag="pb")
                        for k in range(2):
                            nc.tensor.matmul(
                                psb, b3w[:, k, mc * 128 : (mc + 1) * 128],
                                hb2[:, k, :],
                                start=(k == 0), stop=(k == 1),
                            )
                        pr = prp.tile([128, NB], F32R, tag="prod")
                        nc.vector.scalar_tensor_tensor(
                            out=pr[:], in0=psb[:], scalar=b3b[:, mc : mc + 1],
                            in1=u_t[:], op0=ALU.add, op1=ALU.mult,
                        )
                        prods.append(pr)
                    pbu = pmid.tile([Z, NB], F32, tag="pm")
                    for mc in range(4):
                        nc.tensor.matmul(
                            pbu, segt[:, mc * 32 : (mc + 1) * 32], prods[mc],
                            start=(mc == 0), stop=(mc == 3),
                        )

                    # z_next = G0*Z0 + G1*Z1 + DT*Bu   (in-place DVE chain)
                    nc.vector.tensor_tensor(
                        out=p_t[:], in0=f_t[:], in1=p_t[:], op=ALU.mult
                    )
                    nc.vector.tensor_scalar(
                        out=p_t[:], in0=p_t[:], scalar1=DT, scalar2=m0b,
                        op0=ALU.mult, op1=ALU.add,
                    )
                    nc.vector.tensor_tensor(
                        out=q_t[:], in0=f_t[:], in1=q_t[:], op=ALU.mult
                    )
                    nc.vector.tensor_scalar(
                        out=q_t[:], in0=q_t[:], scalar1=DT, scalar2=m1b,
                        op0=ALU.mult, op1=ALU.add,
                    )
                    nc.vector.tensor_tensor(
                        out=p_t[:], in0=p_t[:], in1=z0_t[:], op=ALU.mult
                    )
                    nc.vector.tensor_tensor(
                        out=q_t[:], in0=q_t[:], in1=z1_t[:], op=ALU.mult
                    )
                    nc.vector.tensor_tensor(
                        out=p_t[:], in0=p_t[:], in1=q_t[:], op=ALU.add
                    )
                    zn_t = znp.tile([128, NB], F32R, tag="zn")
                    nc.vector.scalar_tensor_tensor(
                        out=zn_t[:Z], in0=pbu[:], scalar=DT, in1=p_t[:],
                        op0=ALU.mult, op1=ALU.add,
                    )
                    # replicate to all 4 row-groups for quad-packed d1
                    nc.vector.tensor_copy(out=zn_t[Z : 2 * Z], in_=zn_t[:Z])
                    nc.vector.tensor_copy(out=zn_t[2 * Z :], in_=zn_t[: 2 * Z])
                    zn_tiles.append(zn_t)

                # phase B: decoders, pipelined behind phase A on PE
                for c in range(NCHUNK):
                    cs = c * NB
                    zn_t = zn_tiles[c]
                    hd1 = hp.tile([128, 8, NB], F32R, tag="h")
                    for g in range(2):
                        pss = [pbig.tile([128, NB], F32, tag="pb", name=f"d1ps{_r}") for _r in range(4)]
                        for r in range(4):
                            nc.tensor.matmul(
                                pss[r],
                                d1w[32 * r : 32 * r + 32, g * 128 : (g + 1) * 128],
                                zn_t[32 * r : 32 * r + 32, :],
                                start=True, stop=True,
                                tile_position=(32 * r, 0),
                            )
                        for r in range(4):
                            mi = 4 * g + r
                            nc.scalar.activation(
                                hd1[:, mi, :], pss[r], AF.Silu,
                                bias=d1b[:, mi : mi + 1], scale=1.0,
                            )
                    hd2 = hp.tile([128, 8, NB], F32R, tag="h")
                    mlp_layer(d2w, 8, d2b, lambda k: hd1[:, k, :], hd2, 8)
                    hd3 = hp.tile([128, 8, NB], F32R, tag="h")
                    mlp_layer(d3w, 8, d3b, lambda k: hd2[:, k, :], hd3, 8)

                    py_t = pyp.tile([X, NB], F32, tag="py")
                    for k in range(8):
                        nc.tensor.matmul(
                            py_t, d4w[:, k, :], hd3[:, k, :],
                            start=(k == 0), stop=(k == 7),
                        )
                    y_sb = yp.tile([X, NB], F32, tag="y")
                    nc.vector.tensor_scalar_add(
                        out=y_sb[:], in0=py_t[:], scalar1=d4b
                    )
                    nc.sync.dma_start(out=yT[:, cs : cs + NB], in_=y_sb)

    nc.compile()
    return nc


def _prep_host(inputs):
    f32 = np.float32
    x = np.asarray(inputs["x"], f32)
    u = np.asarray(inputs["u"], f32)

    xT = np.ascontiguousarray(x.T)
    x2T = np.concatenate([xT, xT], axis=0)  # [128, B]: x twice (row packing)
    uR = np.tile(np.ascontiguousarray(u.T), (8, 1))  # [128, B]

    def fm(w):
        """[K, M] -> [128, (K//128)*M]: per-partition-contiguous lhsT chunks."""
        kc = w.shape[0] // 128
        return np.ascontiguousarray(
            w.reshape(kc, 128, w.shape[1]).transpose(1, 0, 2).reshape(128, -1)
        )

    def pack_pairs(w):
        """[64, M] -> [128, M//... ] row-packed pairs of 128-col chunks."""
        mt = w.shape[1] // 256
        out = np.zeros((128, mt, 128), f32)
        for j in range(mt):
            out[:64, j] = w[:, (2 * j) * 128 : (2 * j + 1) * 128]
            out[64:, j] = w[:, (2 * j + 1) * 128 : (2 * j + 2) * 128]
        return out

    idx0 = np.arange(Z) // 2 * 2
    idx1 = idx0 + 1
    even = (np.arange(Z) % 2 == 0).astype(f32)

    e_w3 = np.asarray(inputs["e_w3"], f32)
    e_b3 = np.asarray(inputs["e_b3"], f32)
    a_w3 = np.asarray(inputs["a_w3"], f32)
    a_b3 = np.asarray(inputs["a_b3"], f32)

    segw = np.zeros((128, 128), f32)
    for mc in range(4):
        for k in range(128):
            segw[k, mc * 32 + 8 * mc + k // 16] = 1.0

    pi = np.pi

    wpack = np.zeros((128, WCOLS), f32)
    wpack[:, OFF["e1"] : OFF["e1"] + 512] = pack_pairs(
        np.asarray(inputs["e_w1"], f32)
    ).reshape(128, 512)
    wpack[:, OFF["a1"] : OFF["a1"] + 128] = pack_pairs(
        np.asarray(inputs["a_w1"], f32)
    )[:, 0]
    wpack[:, OFF["b1"] : OFF["b1"] + 128] = pack_pairs(
        np.asarray(inputs["b_w1"], f32)
    )[:, 0]
    wpack[:, OFF["a2"] : OFF["a2"] + 512] = fm(np.asarray(inputs["a_w2"], f32))
    wpack[:, OFF["b2"] : OFF["b2"] + 512] = fm(np.asarray(inputs["b_w2"], f32))
    wpack[:, OFF["fpq"] : OFF["fpq"] + 128] = fm(
        np.concatenate([a_w3[:, idx0], a_w3[:, idx1]], axis=1)
    )
    wpack[:, OFF["b3"] : OFF["b3"] + 1024] = fm(np.asarray(inputs["b_w3"], f32))
    wpack[:, OFF["z01"] : OFF["z01"] + 512] = fm(
        np.concatenate([e_w3[:, idx0], e_w3[:, idx1]], axis=1)
    )
    wpack[:, OFF["seg"] : OFF["seg"] + 128] = segw
    wpack[:, OFF["d4"] : OFF["d4"] + 512] = fm(np.asarray(inputs["d_w4"], f32))
    d_w1 = np.asarray(inputs["d_w1"], f32)
    for g in range(2):
        for r in range(4):
            m = 4 * g + r
            wpack[32 * r : 32 * r + 32, OFF["d1"] + g * 128 : OFF["d1"] + (g + 1) * 128] = (
                d_w1[:, m * 128 : (m + 1) * 128]
            )

    def bcol(b):
        return np.asarray(b, f32).reshape(-1, 128).T

    bpack = np.zeros((128, BCOLS), f32)
    bpack[:, 0:8] = bcol(inputs["e_b1"])
    bpack[:, 8:16] = bcol(inputs["e_b2"])
    bpack[:, 16:18] = bcol(inputs["a_b1"])
    bpack[:, 18:20] = bcol(inputs["a_b2"])
    bpack[:, 20:22] = bcol(inputs["b_b1"])
    bpack[:, 22:24] = bcol(inputs["b_b2"])
    bpack[:, 24:28] = bcol(inputs["b_b3"])
    bpack[:, 28:36] = bcol(inputs["d_b1"])
    bpack[:, 36:44] = bcol(inputs["d_b2"])
    bpack[:, 44:52] = bcol(inputs["d_b3"])
    bpack[:64, 52] = np.concatenate([e_b3[idx0], e_b3[idx1]])
    bpack[:32, 53] = DT * a_b3[idx0]
    bpack[32:64, 54] = DT * a_b3[idx1] + even * (pi / 2)
    bpack[32:64, 55] = DT * a_b3[idx1] + np.where(even, pi, pi / 2)
    bpack[:32, 56] = even
    bpack[:32, 57] = 1.0 - even
    bpack[:64, 58] = np.asarray(inputs["d_b4"], f32)

    shared = {
        "wpack": wpack,
        "bpack": bpack,
        "w_e2": fm(np.asarray(inputs["e_w2"], f32)),
        "w_d2": fm(np.asarray(inputs["d_w2"], f32)),
        "w_d3": fm(np.asarray(inputs["d_w3"], f32)),
    }

    in_maps = []
    for c in range(N_CORES):
        sl = slice(c * BC, (c + 1) * BC)
        m = dict(shared)
        m["x2T"] = np.ascontiguousarray(x2T[:, sl])
        m["uR"] = np.ascontiguousarray(uR[:, sl])
        in_maps.append(m)
    return in_maps


def kernel(**inputs) -> np.ndarray:
    from concourse import bass_utils

    if "nc" not in _CACHE:
        _CACHE["nc"] = _build()
    nc = _CACHE["nc"]
    in_maps = _prep_host(inputs)
    res = bass_utils.run_bass_kernel_spmd(
        nc, in_maps, core_ids=list(range(N_CORES))
    )
    return np.concatenate(
        [np.asarray(res.results[c]["yT"]).T for c in range(N_CORES)], axis=0
    ).astype(np.float32)
